# revision 9
# baseline (speedup 1.0000x reference)
"""ComplexUnPooling2D scatter kernel for 8 Trainium2 NeuronCores.

Reference semantics: out_flat = zeros(4*n); out_flat[unpool_mat.ravel()] = inputs.ravel()
where unpool_mat[i] = 4*i + off_i, off_i in [0,4)  (2x2 maxpool argmax structure,
indices strictly increasing, batch-local).  Hence, viewing the output as [n, 4]:

    out[i, j] = inputs[i] * ((unpool_mat[i] & 3) == j)

which is a pure streaming elementwise op — no indirect scatter needed.

Sharding: batch dim across 8 cores (2 batches/core).  Each core reads its input
slice (f32) + index slice (int64 viewed as int32 pairs; only low words used
on-device) and writes its 4x-sized output slice.  Host does only slicing /
reinterpret-cast / concatenation.
"""
import sys

sys.path.insert(0, "/opt/trn_rl_repo")

import numpy as np

import concourse.bacc as bacc
import concourse.mybir as mybir
import concourse.tile as tile
from concourse.bass_utils import run_bass_kernel_spmd

# Problem constants (hardcoded per contract)
B, H, W, C = 16, 64, 64, 128
OUT_SHAPE = (B, 2 * H, 2 * W, C)
N_CORES = 8
N_PER_CORE = (B // N_CORES) * H * W * C  # 1,048,576 elements
P = 128  # SBUF partitions

# Tiling: input viewed per-core as [T*P, F]
F = 1024
T = N_PER_CORE // (P * F)  # 8
assert T * P * F == N_PER_CORE


def _build_program():
    # Bacc (not raw Bass): its compile() runs generate_event_semaphores,
    # which splits multi-sem waits (TRN2 allows max 1 wait per instruction).
    nc = bacc.Bacc(
        "TRN2",
        target_bir_lowering=False,
        debug=False,
        num_devices=N_CORES,
    )
    x = nc.dram_tensor("x", [T * P, F], mybir.dt.float32, kind="ExternalInput").ap()
    idx32 = nc.dram_tensor(
        "idx32", [T * P, 2 * F], mybir.dt.int32, kind="ExternalInput"
    ).ap()
    y = nc.dram_tensor("y", [T * P, 4 * F], mybir.dt.float32, kind="ExternalOutput").ap()

    AL = mybir.AluOpType
    with tile.TileContext(nc) as tc:
        with tc.tile_pool(name="pool", bufs=4) as pool:
            for t in range(T):
                rows = slice(t * P, (t + 1) * P)
                xt = pool.tile([P, F], mybir.dt.float32, tag="x")
                it = pool.tile([P, 2 * F], mybir.dt.int32, tag="idx")
                ot = pool.tile([P, 4 * F], mybir.dt.float32, tag="out")
                off = pool.tile([P, F], mybir.dt.int32, tag="off")
                m2 = pool.tile([P, F], mybir.dt.float32, tag="m2")
                m3 = pool.tile([P, F], mybir.dt.float32, tag="m3")
                nc.sync.dma_start(out=xt[:], in_=x[rows, :])
                nc.sync.dma_start(out=it[:], in_=idx32[rows, :])
                # low int32 word of each little-endian int64 index
                lo = it.rearrange("p (f two) -> p f two", two=2)[:, :, 0]
                # off = lo & 3  (int-domain; bitwise ops can't cast on write)
                nc.vector.tensor_scalar(
                    out=off[:], in0=lo, scalar1=3, scalar2=None,
                    op0=AL.bitwise_and,
                )
                o4 = ot.rearrange("p (f four) -> p f four", four=4)
                # j=0,1: fused (off==j)*x on DVE, strided write
                for j in range(2):
                    nc.vector.scalar_tensor_tensor(
                        out=o4[:, :, j], in0=off[:], scalar=float(j), in1=xt[:],
                        op0=AL.is_equal, op1=AL.mult,
                    )
                # j=2,3: DVE computes masks contiguously; gpsimd multiplies
                # into the strided slots — splits work across both engines.
                for j, m in ((2, m2), (3, m3)):
                    nc.vector.tensor_scalar(
                        out=m[:], in0=off[:], scalar1=j, scalar2=None,
                        op0=AL.is_equal,
                    )
                    nc.gpsimd.tensor_tensor(
                        out=o4[:, :, j], in0=m[:], in1=xt[:], op=AL.mult,
                    )
                nc.sync.dma_start(out=y[rows, :], in_=ot[:])
    nc.compile()
    return nc


_NC_CACHE = None


def _get_program():
    global _NC_CACHE
    if _NC_CACHE is None:
        _NC_CACHE = _build_program()
    return _NC_CACHE


def _index_pairs_int32(unpool_mat: np.ndarray) -> np.ndarray:
    """Per-core int64 index slice -> [T*P, 2F] int32 (little-endian lo/hi pairs)."""
    idx = np.ascontiguousarray(unpool_mat).reshape(-1)
    if idx.dtype == np.int64:
        return idx.view(np.int32).reshape(T * P, 2 * F)
    # int32 fallback: interleave with zero high words (layout-only host work)
    pairs = np.zeros((idx.size, 2), dtype=np.int32)
    pairs[:, 0] = idx.astype(np.int32, copy=False)
    return pairs.reshape(T * P, 2 * F)


def kernel(inputs, unpool_mat, output_shape=None, **_unused):
    inputs = np.asarray(inputs)
    unpool_mat = np.asarray(unpool_mat)
    assert inputs.shape == (B, H, W, C), inputs.shape
    if output_shape is not None:
        assert tuple(int(s) for s in np.asarray(output_shape).reshape(-1)) == OUT_SHAPE

    nc = _get_program()
    bpc = B // N_CORES  # batches per core
    in_maps = []
    for c in range(N_CORES):
        sl = slice(c * bpc, (c + 1) * bpc)
        in_maps.append(
            {
                "x": np.ascontiguousarray(inputs[sl]).reshape(T * P, F),
                "idx32": _index_pairs_int32(unpool_mat[sl]),
            }
        )

    res = run_bass_kernel_spmd(nc, in_maps, core_ids=list(range(N_CORES)))
    out = np.concatenate(
        [r["y"].reshape(bpc, 2 * H, 2 * W, C) for r in res.results], axis=0
    )
    return out


# revision 10
# speedup vs baseline: 1.1219x; 1.1219x over previous
"""ComplexUnPooling2D scatter kernel for 8 Trainium2 NeuronCores.

Reference semantics: out_flat = zeros(4*n); out_flat[unpool_mat.ravel()] = inputs.ravel()
where unpool_mat[i] = 4*i + off_i, off_i in [0,4)  (2x2 maxpool argmax structure,
indices strictly increasing, batch-local).  Hence, viewing the output as [n, 4]:

    out[i, j] = inputs[i] * ((unpool_mat[i] & 3) == j)

which is a pure streaming elementwise op — no indirect scatter needed.

Sharding: batch dim across 8 cores (2 batches/core).  Each core reads its input
slice (f32) + index slice (int64 viewed as int32 pairs; only low words used
on-device) and writes its 4x-sized output slice.  Host does only slicing /
reinterpret-cast / concatenation.
"""
import sys

sys.path.insert(0, "/opt/trn_rl_repo")

import numpy as np

import concourse.bacc as bacc
import concourse.mybir as mybir
import concourse.tile as tile
from concourse.bass_utils import run_bass_kernel_spmd

# Problem constants (hardcoded per contract)
B, H, W, C = 16, 64, 64, 128
OUT_SHAPE = (B, 2 * H, 2 * W, C)
N_CORES = 8
N_PER_CORE = (B // N_CORES) * H * W * C  # 1,048,576 elements
P = 128  # SBUF partitions

# Tiling: input viewed per-core as [T*P, F]
F = 1024
T = N_PER_CORE // (P * F)  # 8
assert T * P * F == N_PER_CORE


def _build_program():
    # Bacc (not raw Bass): its compile() runs generate_event_semaphores,
    # which splits multi-sem waits (TRN2 allows max 1 wait per instruction).
    nc = bacc.Bacc(
        "TRN2",
        target_bir_lowering=False,
        debug=False,
        num_devices=N_CORES,
    )
    x = nc.dram_tensor("x", [T * P, F], mybir.dt.float32, kind="ExternalInput").ap()
    idx32 = nc.dram_tensor(
        "idx32", [T * P, 2 * F], mybir.dt.int32, kind="ExternalInput"
    ).ap()
    y = nc.dram_tensor("y", [T * P, 4 * F], mybir.dt.float32, kind="ExternalOutput").ap()

    AL = mybir.AluOpType
    with tile.TileContext(nc) as tc:
        with (
            tc.tile_pool(name="pin", bufs=6) as pin,
            tc.tile_pool(name="pout", bufs=5) as pout,
        ):
            for t in range(T):
                rows = slice(t * P, (t + 1) * P)
                xt = pin.tile([P, F], mybir.dt.float32, tag="x")
                it = pin.tile([P, 2 * F], mybir.dt.int32, tag="idx")
                off = pin.tile([P, F], mybir.dt.int32, tag="off")
                ot = pout.tile([P, 4 * F], mybir.dt.float32, tag="out")
                nc.sync.dma_start(out=xt[:], in_=x[rows, :])
                nc.sync.dma_start(out=it[:], in_=idx32[rows, :])
                # low int32 word of each little-endian int64 index
                lo = it.rearrange("p (f two) -> p f two", two=2)[:, :, 0]
                # off = lo & 3  (int-domain; bitwise ops can't cast on write)
                nc.vector.tensor_scalar(
                    out=off[:], in0=lo, scalar1=3, scalar2=None,
                    op0=AL.bitwise_and,
                )
                o4 = ot.rearrange("p (f four) -> p f four", four=4)
                for j in range(4):
                    # o4[:,:,j] = (off == j) * x   — fused DVE op
                    nc.vector.scalar_tensor_tensor(
                        out=o4[:, :, j], in0=off[:], scalar=float(j), in1=xt[:],
                        op0=AL.is_equal, op1=AL.mult,
                    )
                nc.sync.dma_start(out=y[rows, :], in_=ot[:])
    nc.compile()
    return nc


_NC_CACHE = None


def _get_program():
    global _NC_CACHE
    if _NC_CACHE is None:
        _NC_CACHE = _build_program()
    return _NC_CACHE


def _index_pairs_int32(unpool_mat: np.ndarray) -> np.ndarray:
    """Per-core int64 index slice -> [T*P, 2F] int32 (little-endian lo/hi pairs)."""
    idx = np.ascontiguousarray(unpool_mat).reshape(-1)
    if idx.dtype == np.int64:
        return idx.view(np.int32).reshape(T * P, 2 * F)
    # int32 fallback: interleave with zero high words (layout-only host work)
    pairs = np.zeros((idx.size, 2), dtype=np.int32)
    pairs[:, 0] = idx.astype(np.int32, copy=False)
    return pairs.reshape(T * P, 2 * F)


def kernel(inputs, unpool_mat, output_shape=None, **_unused):
    inputs = np.asarray(inputs)
    unpool_mat = np.asarray(unpool_mat)
    assert inputs.shape == (B, H, W, C), inputs.shape
    if output_shape is not None:
        assert tuple(int(s) for s in np.asarray(output_shape).reshape(-1)) == OUT_SHAPE

    nc = _get_program()
    bpc = B // N_CORES  # batches per core
    in_maps = []
    for c in range(N_CORES):
        sl = slice(c * bpc, (c + 1) * bpc)
        in_maps.append(
            {
                "x": np.ascontiguousarray(inputs[sl]).reshape(T * P, F),
                "idx32": _index_pairs_int32(unpool_mat[sl]),
            }
        )

    res = run_bass_kernel_spmd(nc, in_maps, core_ids=list(range(N_CORES)))
    out = np.concatenate(
        [r["y"].reshape(bpc, 2 * H, 2 * W, C) for r in res.results], axis=0
    )
    return out


# revision 11
# speedup vs baseline: 1.2713x; 1.1332x over previous
"""ComplexUnPooling2D scatter kernel for 8 Trainium2 NeuronCores.

Reference semantics: out_flat = zeros(4*n); out_flat[unpool_mat.ravel()] = inputs.ravel()
where unpool_mat[i] = 4*i + off_i, off_i in [0,4)  (2x2 maxpool argmax structure,
indices strictly increasing, batch-local).  Hence, viewing the output as [n, 4]:

    out[i, j] = inputs[i] * ((unpool_mat[i] & 3) == j)

which is a pure streaming elementwise op — no indirect scatter needed.

Sharding: batch dim across 8 cores (2 batches/core).  Each core reads its input
slice (f32) + index slice (int64 viewed as int32 pairs; only low words used
on-device) and writes its 4x-sized output slice.  Host does only slicing /
reinterpret-cast / concatenation.
"""
import sys

sys.path.insert(0, "/opt/trn_rl_repo")

import numpy as np

import concourse.bacc as bacc
import concourse.mybir as mybir
import concourse.tile as tile
from concourse.bass_utils import run_bass_kernel_spmd

# Problem constants (hardcoded per contract)
B, H, W, C = 16, 64, 64, 128
OUT_SHAPE = (B, 2 * H, 2 * W, C)
N_CORES = 8
N_PER_CORE = (B // N_CORES) * H * W * C  # 1,048,576 elements
P = 128  # SBUF partitions

# Tiling: input viewed per-core as [T*P, F]
F = 1024
T = N_PER_CORE // (P * F)  # 8
assert T * P * F == N_PER_CORE


def _build_program():
    # Bacc (not raw Bass): its compile() runs generate_event_semaphores,
    # which splits multi-sem waits (TRN2 allows max 1 wait per instruction).
    nc = bacc.Bacc(
        "TRN2",
        target_bir_lowering=False,
        debug=False,
        num_devices=N_CORES,
    )
    x = nc.dram_tensor("x", [T * P, F], mybir.dt.float32, kind="ExternalInput").ap()
    idx32 = nc.dram_tensor(
        "idx32", [T * P, 2 * F], mybir.dt.int32, kind="ExternalInput"
    ).ap()
    y = nc.dram_tensor("y", [T * P, 4 * F], mybir.dt.float32, kind="ExternalOutput").ap()

    AL = mybir.AluOpType
    with tile.TileContext(nc) as tc:
        with (
            tc.tile_pool(name="pin", bufs=6) as pin,
            tc.tile_pool(name="pout", bufs=5) as pout,
        ):
            for t in range(T):
                rows = slice(t * P, (t + 1) * P)
                xt = pin.tile([P, F], mybir.dt.float32, tag="x")
                it = pin.tile([P, 2 * F], mybir.dt.int32, tag="idx")
                off = pin.tile([P, F], mybir.dt.int32, tag="off")
                ot = pout.tile([P, 4 * F], mybir.dt.float32, tag="out")
                # inputs on the Activation-engine HWDGE ring, outputs on the
                # sync ring — separate queue sets, so input loads don't queue
                # behind multi-MB output stores (head-of-line blocking).
                nc.scalar.dma_start(out=xt[:], in_=x[rows, :])
                nc.scalar.dma_start(out=it[:], in_=idx32[rows, :])
                # low int32 word of each little-endian int64 index
                lo = it.rearrange("p (f two) -> p f two", two=2)[:, :, 0]
                # off = lo & 3  (int-domain; bitwise ops can't cast on write)
                nc.vector.tensor_scalar(
                    out=off[:], in0=lo, scalar1=3, scalar2=None,
                    op0=AL.bitwise_and,
                )
                o4 = ot.rearrange("p (f four) -> p f four", four=4)
                for j in range(4):
                    # o4[:,:,j] = (off == j) * x   — fused DVE op
                    nc.vector.scalar_tensor_tensor(
                        out=o4[:, :, j], in0=off[:], scalar=float(j), in1=xt[:],
                        op0=AL.is_equal, op1=AL.mult,
                    )
                nc.sync.dma_start(out=y[rows, :], in_=ot[:])
    nc.compile()
    return nc


_NC_CACHE = None


def _get_program():
    global _NC_CACHE
    if _NC_CACHE is None:
        _NC_CACHE = _build_program()
    return _NC_CACHE


def _index_pairs_int32(unpool_mat: np.ndarray) -> np.ndarray:
    """Per-core int64 index slice -> [T*P, 2F] int32 (little-endian lo/hi pairs)."""
    idx = np.ascontiguousarray(unpool_mat).reshape(-1)
    if idx.dtype == np.int64:
        return idx.view(np.int32).reshape(T * P, 2 * F)
    # int32 fallback: interleave with zero high words (layout-only host work)
    pairs = np.zeros((idx.size, 2), dtype=np.int32)
    pairs[:, 0] = idx.astype(np.int32, copy=False)
    return pairs.reshape(T * P, 2 * F)


def kernel(inputs, unpool_mat, output_shape=None, **_unused):
    inputs = np.asarray(inputs)
    unpool_mat = np.asarray(unpool_mat)
    assert inputs.shape == (B, H, W, C), inputs.shape
    if output_shape is not None:
        assert tuple(int(s) for s in np.asarray(output_shape).reshape(-1)) == OUT_SHAPE

    nc = _get_program()
    bpc = B // N_CORES  # batches per core
    in_maps = []
    for c in range(N_CORES):
        sl = slice(c * bpc, (c + 1) * bpc)
        in_maps.append(
            {
                "x": np.ascontiguousarray(inputs[sl]).reshape(T * P, F),
                "idx32": _index_pairs_int32(unpool_mat[sl]),
            }
        )

    res = run_bass_kernel_spmd(nc, in_maps, core_ids=list(range(N_CORES)))
    out = np.concatenate(
        [r["y"].reshape(bpc, 2 * H, 2 * W, C) for r in res.results], axis=0
    )
    return out
